# revision 1
# baseline (speedup 1.0000x reference)
"""Trainium2 Bass kernel for sparse_attention problem nn_CAMD_73229192397362.

All-fp32 (pointwise-rel-faithful to the fp32 reference). Speed comes from
scheduling, not dtype:
  - PE sub-tile concurrency: stacked MLPs on the quadrant diagonal
    (tile_position=(32k,32k)); band S^T matmuls ride the natural quadrant
    rotation of the stacked K^T layout (tile_position=(32k,0)); zo/prefix
    matmuls rotate PE column position by query tile (tile_position=(0,32t))
    writing disjoint PSUM partition pairs.
  - Chunk-major banding: per key chunk ONE S^T matmul + ONE fused mask
    (scalar_tensor_tensor) spanning all query tiles that touch the chunk
    (grouped in 4-tile windows) -> ~75 DVE mask ops instead of 230+.
  - The running prefix state H stays fp32; per-tile H snapshots feed
    per-tile prefix matmuls accumulated in the same PSUM as the band.

Per core (8 cores = 4 modalities x 2 interleaved query half-sets so band
metadata is uniform): Q = MLP(m1) (4096 local queries), K = MLP(m_c),
Z[i] = Q_i . H(w_I) + sum_{j in band, t2[j] <= t1[i]} (Q_i.K_j) V2_j.
"""

import numpy as np

import concourse.bass as bass
from concourse.bacc import Bacc
import concourse.mybir as mybir
from concourse.tile import TileContext
from concourse.bass_utils import run_bass_kernel_spmd

T = 8192
D = 32
TQ = 4096          # queries per core
NT = TQ // 128     # query tiles per core (32)
NCH = T // 128     # key chunks (64)
NG = NT // 4       # zo groups (8)
F32 = mybir.dt.float32
AF = mybir.ActivationFunctionType
OP = mybir.AluOpType


def _stack4(xT):
    """(32, Ttot) -> (128, Ttot//4): 512-col chunk g goes to partition
    block g%4, columns (g//4)*512."""
    d, Ttot = xT.shape
    ng = Ttot // 512
    out = np.zeros((128, Ttot // 4), dtype=xT.dtype)
    for g in range(ng):
        k = g % 4
        out[32 * k:32 * k + 32, (g // 4) * 512:(g // 4) * 512 + 512] = \
            xT[:, g * 512:(g + 1) * 512]
    return out


def _band_meta(t1_all, t2_all):
    """Uniform band metadata. Returns (w, e): per tile the 128-aligned
    band start chunk wc[I]=w[I]//128 and exclusive end chunk e[I], with
    both sequences monotone nondecreasing."""
    w_raw = np.full(NT, T, dtype=np.int64)
    for t1 in t1_all:
        for t2 in t2_all:
            r_min = np.searchsorted(t2, t1[::128], side="right")
            w_raw = np.minimum(w_raw, (r_min // 128) * 128)
    e = np.zeros(NT, dtype=np.int64)
    for t1 in t1_all:
        for t2 in t2_all:
            r_max = np.searchsorted(t2, t1[127::128], side="right")
            e = np.maximum(e, (r_max + 127) // 128)
    wc = w_raw // 128
    e = np.maximum(e, wc + 1)
    e = np.minimum(np.maximum.accumulate(e), NCH)
    wc = np.minimum(wc, e - 1)
    assert np.all(np.diff(wc) >= 0) and np.all(np.diff(e) >= 0)
    for t1 in t1_all:
        for t2 in t2_all:
            r_min = np.searchsorted(t2, t1[::128], side="right")
            r_max = np.searchsorted(t2, t1[127::128], side="right")
            assert np.all(wc * 128 <= r_min) and np.all(r_max <= e * 128)
    return [int(x) for x in wc], [int(x) for x in e]


def _build(wc, e):
    """Build the Bass module (same program for all 8 cores)."""
    import os as _os
    _phase = int(_os.environ.get("BISECT_PHASE", "9"))
    nc = Bacc("TRN2")

    xk = nc.dram_tensor("xk", [128, T // 4], F32, kind="ExternalInput")
    xq = nc.dram_tensor("xq", [128, TQ // 4], F32, kind="ExternalInput")
    wk = nc.dram_tensor("wk", [128, 96], F32, kind="ExternalInput")
    wq = nc.dram_tensor("wq", [128, 96], F32, kind="ExternalInput")
    bk = nc.dram_tensor("bk", [128, 3], F32, kind="ExternalInput")
    bq = nc.dram_tensor("bq", [128, 3], F32, kind="ExternalInput")
    id4 = nc.dram_tensor("id4", [128, 32], F32, kind="ExternalInput")
    t1b = nc.dram_tensor("t1b", [128, TQ], F32, kind="ExternalInput")
    t2p = nc.dram_tensor("t2p", [128, NCH], F32, kind="ExternalInput")
    v2n = nc.dram_tensor("v2n", [128, 2 * NCH], F32, kind="ExternalInput")
    out = nc.dram_tensor("out", [2, TQ], F32, kind="ExternalOutput")

    # host-side chunk geometry --------------------------------------------
    def quad(c):          # PE quadrant of chunk c in the stacked layout
        return (c // 4) % 4

    def ktcol(c):         # kt_s column of chunk c
        return (c // 16) * 512 + (c % 4) * 128

    # group-local band pieces: (c, Ilo, Ihi) with [Ilo,Ihi) inside group g
    pieces = {g: [] for g in range(NG)}
    for c in range(min(wc), max(e)):
        ilo = next((i for i in range(NT) if wc[i] <= c < e[i]), None)
        if ilo is None:
            continue
        ihi = max(i for i in range(NT) if wc[i] <= c < e[i]) + 1
        for i in range(ilo, ihi):
            assert wc[i] <= c < e[i]
        g0, g1 = ilo // 4, (ihi - 1) // 4
        for g in range(g0, g1 + 1):
            lo, hi = max(ilo, 4 * g), min(ihi, 4 * g + 4)
            pieces[g].append((c, lo, hi))
    # quadrant round-robin order within each group
    for g in range(NG):
        by_q = [[p for p in pieces[g] if quad(p[0]) == q] for q in range(4)]
        order = []
        while any(by_q):
            for q in range(4):
                if by_q[q]:
                    order.append(by_q[q].pop(0))
        pieces[g] = order
    max_pieces = max(len(pieces[g]) for g in range(NG))

    with TileContext(nc) as tc:
        with tc.tile_pool(name="cst", bufs=1) as cst, \
             tc.tile_pool(name="big", bufs=1) as big, \
             tc.tile_pool(name="hps", bufs=2, space="PSUM") as hps:

            wk_s = cst.tile([128, 96], F32)
            wq_s = cst.tile([128, 96], F32)
            bk_s = cst.tile([128, 3], F32)
            bq_s = cst.tile([128, 3], F32)
            id4_s = cst.tile([128, 32], F32)
            t1b_s = big.tile([128, TQ], F32, tag="t1b")
            t2p_s = cst.tile([128, NCH], F32)
            v2n_s = cst.tile([128, 2 * NCH], F32)
            xk_s = big.tile([128, T // 4], F32, tag="xk")
            xq_s = big.tile([128, TQ // 4], F32, tag="xq")
            kt_s = big.tile([128, T // 4], F32, tag="kt")   # K^T stacked
            ktf = big.tile([32, T], F32, tag="ktf")          # K^T flat
            knat = big.tile([128, NCH * 32], F32, tag="knat")
            qtf4 = big.tile([32, TQ], F32, tag="qtf4")       # Q^T flat
            zsb4 = cst.tile([128, NG * 128], F32)

            for dst, src in ((wk_s, wk), (wq_s, wq), (bk_s, bk), (bq_s, bq),
                             (id4_s, id4), (t1b_s, t1b), (t2p_s, t2p),
                             (v2n_s, v2n), (xk_s, xk), (xq_s, xq)):
                nc.sync.dma_start(dst[:], src[:])

            # ---------------- MLPs ----------------
            with tc.tile_pool(name="mlp", bufs=3, space="PSUM") as mlp, \
                 tc.tile_pool(name="hbuf", bufs=2) as hbuf:

                def mlp_hidden(x_s, w_s, b_s, ngrp, eng):
                    h_prev = x_s
                    for l in range(2):
                        h_next = hbuf.tile(
                            [128, ngrp * 512], F32, tag=f"h{id(x_s)}")
                        for G in range(ngrp):
                            pt = mlp.tile([128, 512], F32, tag="mlp")
                            for k in range(4):
                                nc.tensor.matmul(
                                    pt[32 * k:32 * k + 32, :],
                                    w_s[32 * k:32 * k + 32, 32 * l:32 * l + 32],
                                    h_prev[32 * k:32 * k + 32,
                                           G * 512:(G + 1) * 512],
                                    start=True, stop=True,
                                    tile_position=(32 * k, 32 * k),
                                )
                            if eng is nc.scalar:
                                eng.activation(
                                    h_next[:, G * 512:(G + 1) * 512], pt[:],
                                    AF.Relu, bias=b_s[:, l:l + 1])
                            else:
                                eng.tensor_scalar(
                                    h_next[:, G * 512:(G + 1) * 512], pt[:],
                                    b_s[:, l:l + 1], 0.0, OP.add, OP.max)
                        h_prev = h_next
                    return h_prev

                h2k = mlp_hidden(xk_s, wk_s, bk_s, 4, nc.vector)
                h2q = mlp_hidden(xq_s, wq_s, bq_s, 2, nc.scalar)

                # K final layer -> stacked kt_s
                for G in range(4):
                    pt = mlp.tile([128, 512], F32, tag="mlp")
                    for k in range(4):
                        nc.tensor.matmul(
                            pt[32 * k:32 * k + 32, :],
                            wk_s[32 * k:32 * k + 32, 64:96],
                            h2k[32 * k:32 * k + 32, G * 512:(G + 1) * 512],
                            start=True, stop=True,
                            tile_position=(32 * k, 32 * k),
                        )
                    if G % 2:
                        nc.scalar.activation(
                            kt_s[:, G * 512:(G + 1) * 512], pt[:],
                            AF.Identity, bias=bk_s[:, 2:3])
                    else:
                        nc.vector.tensor_scalar(
                            kt_s[:, G * 512:(G + 1) * 512], pt[:],
                            bk_s[:, 2:3], None, OP.add)

                # Q final layer -> stacked qts
                qts = hbuf.tile([128, TQ // 4], F32, tag="qts")
                for G in range(2):
                    pt = mlp.tile([128, 512], F32, tag="mlp")
                    for k in range(4):
                        nc.tensor.matmul(
                            pt[32 * k:32 * k + 32, :],
                            wq_s[32 * k:32 * k + 32, 64:96],
                            h2q[32 * k:32 * k + 32, G * 512:(G + 1) * 512],
                            start=True, stop=True,
                            tile_position=(32 * k, 32 * k),
                        )
                    nc.scalar.activation(
                        qts[:, G * 512:(G + 1) * 512], pt[:],
                        AF.Identity, bias=bq_s[:, 2:3])

                # un-stack K and Q to flat (32, x)
                for k in range(4):
                    nc.sync.dma_start(
                        ktf[0:32, :].rearrange(
                            "d (p f) -> d p f", f=512)[:, k::4, :],
                        kt_s[32 * k:32 * k + 32, :])
                for k in range(4):
                    nc.sync.dma_start(
                        qtf4[0:32, :].rearrange(
                            "d (p f) -> d p f", f=512)[:, k::4, :],
                        qts[32 * k:32 * k + 32, :])

                # K natural layout via identity transpose (baseline form)
                for P in range(4):
                    pt = mlp.tile([128, 512], F32, tag="mlp")
                    for j in range(16):
                        c = 16 * P + j
                        nc.tensor.matmul(
                            pt[:, 32 * j:32 * j + 32],
                            ktf[:, 128 * c:128 * c + 128],
                            id4_s[0:32, :],
                            start=True, stop=True,
                        )
                    nc.vector.tensor_copy(
                        knat[:, P * 512:(P + 1) * 512], pt[:])

            # ---------------- attention ----------------
            with tc.tile_pool(name="stp", bufs=4, space="PSUM") as stp, \
                 tc.tile_pool(name="zop", bufs=2, space="PSUM") as zop, \
                 tc.tile_pool(name="smp", bufs=max_pieces + 2) as smp, \
                 tc.tile_pool(name="hsb", bufs=2) as hsbp:

                hsb = hsbp.tile([32, 2], F32, tag="hsb")
                nc.vector.memset(hsb[:], 0)
                delta_done = min(wc)
                if _phase < 1:
                    delta_done = max(e)
                if delta_done > 0 and _phase >= 1:
                    dps = hps.tile([32, 2], F32, tag="dh")
                    for c in range(0, delta_done):
                        nc.tensor.matmul(
                            dps[:], knat[:, 32 * c:32 * c + 32],
                            v2n_s[:, 2 * c:2 * c + 2],
                            start=(c == 0), stop=(c == delta_done - 1))
                    hsb_new = hsbp.tile([32, 2], F32, tag="hsb")
                    nc.vector.tensor_tensor(hsb_new[:], hsb[:], dps[:], OP.add)
                    hsb = hsb_new

                hsb_for = {}
                for g in range(NG):
                    # H snapshots for this group's tiles
                    for t in range(4):
                        I = 4 * g + t
                        if wc[I] > delta_done and _phase >= 1:
                            dps = hps.tile([32, 2], F32, tag="dh")
                            for c in range(delta_done, wc[I]):
                                nc.tensor.matmul(
                                    dps[:], knat[:, 32 * c:32 * c + 32],
                                    v2n_s[:, 2 * c:2 * c + 2],
                                    start=(c == delta_done),
                                    stop=(c == wc[I] - 1))
                            hsb_new = hsbp.tile([32, 2], F32, tag="hsb")
                            nc.vector.tensor_tensor(
                                hsb_new[:], hsb[:], dps[:], OP.add)
                            hsb = hsb_new
                            delta_done = wc[I]
                        hsb_for[I] = hsb

                    # chunk-major band S^T + fused mask over tile spans
                    smt_of = {}
                    for (c, lo, hi) in (pieces[g] if _phase >= 3 else []):
                        wd = 128 * (hi - lo)
                        k = quad(c)
                        stb = stp.tile([128, 512], F32, tag="st")
                        nc.tensor.matmul(
                            stb[:, 0:wd],
                            ktf[:, 128 * c:128 * c + 128],
                            qtf4[0:32, 128 * lo:128 * hi],
                            start=True, stop=True,
                        )
                        smt = smp.tile([128, 512], F32, tag="smt")
                        nc.vector.scalar_tensor_tensor(
                            smt[:, 0:wd],
                            t1b_s[:, 128 * lo:128 * hi],
                            t2p_s[:, c:c + 1],
                            stb[:, 0:wd],
                            OP.is_ge, OP.mult)
                        smt_of[c] = (smt, lo)

                    # per-tile prefix + band zo into rotated PSUM columns
                    zo4 = zop.tile([128, 128], F32, tag="zo")
                    for t in (range(4) if _phase >= 1 else []):
                        I = 4 * g + t
                        nc.tensor.matmul(
                            zo4[32 * t:32 * t + 2, :], hsb_for[I][:],
                            qtf4[0:32, 128 * I:128 * I + 128],
                            start=True, stop=(_phase < 4),
                            tile_position=(0, 32 * t))
                    for t in range(4):
                        I = 4 * g + t
                        for c in (range(wc[I], e[I]) if _phase >= 4 else []):
                            smt, lo = smt_of[c]
                            off = 128 * (I - lo)
                            nc.tensor.matmul(
                                zo4[32 * t:32 * t + 2, :],
                                v2n_s[:, 2 * c:2 * c + 2],
                                smt[:, off:off + 128],
                                start=False, stop=(c == e[I] - 1),
                                tile_position=(0, 32 * t))
                    for t in (range(4) if _phase >= 1 else []):
                        nc.scalar.activation(
                            zsb4[32 * t:32 * t + 2, 128 * g:128 * g + 128],
                            zo4[32 * t:32 * t + 2, :], AF.Copy)

                # stitch (2, TQ): query 128*(4g+t)+c  <-  zsb4[32t+v, 128g+c]
                if _phase >= 2:
                    for t in range(4):
                        nc.sync.dma_start(
                            out[:, :].rearrange(
                                "p (g f c) -> p g f c", f=4, c=128)[:, :, t:t + 1, :],
                            zsb4[32 * t:32 * t + 2, :].rearrange(
                                "p (g c) -> p g c", c=128))
                else:
                    nc.sync.dma_start(out[:], qtf4[0:2, :])
    nc.finalize()
    return nc


_CACHE = {}
LAST_RESULTS = None


def kernel(m1, m2, m3, m4, Wq, bq, Wk, bk):
    mods = [np.asarray(m)[0, 0].astype(np.float32) for m in (m1, m2, m3, m4)]
    Wq, bq, Wk, bk = (np.asarray(a, dtype=np.float32) for a in (Wq, bq, Wk, bk))
    t2s = [m[:, -1].copy() for m in mods]
    t1g = mods[0][:, -1].copy()

    # core c: modality c//2, half h=c%2 takes global query tiles 2I+h
    def qsel(h):
        idx = np.arange(TQ)
        gt = 2 * (idx // 128) + h
        return gt * 128 + (idx % 128)

    sels = [qsel(0), qsel(1)]
    t1_locals = [t1g[s] for s in sels]
    wc, e = _band_meta(t1_locals, t2s)

    key = (tuple(wc), tuple(e))
    if key not in _CACHE:
        _CACHE[key] = _build(wc, e)
    nc = _CACHE[key]

    wq_in = np.tile(np.concatenate([Wq[l] for l in range(3)], axis=1), (4, 1))
    wk_in = np.tile(np.concatenate([Wk[l] for l in range(3)], axis=1), (4, 1))
    bq_in = np.tile(bq.T, (4, 1)).astype(np.float32)
    bk_in = np.tile(bk.T, (4, 1)).astype(np.float32)
    id4_in = np.tile(np.eye(32, dtype=np.float32), (4, 1))

    in_maps = []
    for c in range(8):
        mod, h = c // 2, c % 2
        x = mods[mod]
        t2 = t2s[mod]
        xk_in = _stack4(np.ascontiguousarray(x.T))
        xq_l = mods[0][sels[h]]
        xq_in = _stack4(np.ascontiguousarray(xq_l.T))
        t1b_in = np.ascontiguousarray(
            np.broadcast_to(t1_locals[h], (128, TQ))).astype(np.float32)
        t2p_in = np.ascontiguousarray(t2.reshape(NCH, 128).T)
        v2n_in = np.ascontiguousarray(
            x[:, :2].reshape(NCH, 128, 2).transpose(1, 0, 2)
            .reshape(128, 2 * NCH))
        in_maps.append({
            "xk": xk_in, "xq": xq_in, "wk": wk_in, "wq": wq_in,
            "bk": bk_in, "bq": bq_in, "id4": id4_in, "t1b": t1b_in,
            "t2p": t2p_in, "v2n": v2n_in,
        })

    import os as _os
    trace = bool(_os.environ.get("KERNEL_TRACE"))
    res = run_bass_kernel_spmd(nc, in_maps, core_ids=list(range(8)),
                               trace=trace)
    global LAST_RESULTS
    LAST_RESULTS = res

    y = np.zeros((T, 2), dtype=np.float32)
    for c in range(8):
        mod, h = c // 2, c % 2
        zt = res.results[c]["out"]          # (2, TQ) local order
        y[sels[h]] += zt.T
    return y[None, :, :]



# revision 16
# speedup vs baseline: 1.5316x; 1.5316x over previous
"""Trainium2 Bass kernel for sparse_attention problem nn_CAMD_73229192397362.

v4 precision model (HW-validated: fp32r = round-to-nearest ~11-bit, but
the tolerance needs ~15+ bits on every path feeding the 7e4-magnitude
accumulations):
  - Both MLPs, band S^T, knat transposes, H snapshots and prefix-zo run
    in fp32.
  - The prefix chain K^T V runs as THREE fp32r matmuls per chunk
    (Kr Vr + Kr Ve + Ke Vr) with exact splits: V split on host,
    K split on-chip from the fp32 knat (round-copy + subtract).
  - The band zo runs fp32r on the fp32-exact masked S (smt) and Vr; its
    residuals are per-key random and average out over the band.

Structure per core (8 = 4 modalities x 2 query half-sets):
  stacked s4 layout (chunk c -> partitions 32*(c%4), cols 128*(c//4));
  block-diag 128-contract MLPs; rotated (tile_position) 32-contract
  band S^T and knat transposes; per-tile prefix H folded in via rotated
  fp32 prefix-zo into 4 PSUM banks (zoP), combined with the band zo
  accumulator (zoB) on DVE at the end of each 512-query group.
"""

import numpy as np

import concourse.bass as bass
from concourse.bacc import Bacc
import concourse.mybir as mybir
from concourse.tile import TileContext
from concourse.bass_utils import run_bass_kernel_spmd

T = 8192
D = 32
TQ = 4096
NT = TQ // 128
NCH = T // 128
NG = NT // 4
F32 = mybir.dt.float32
F32R = mybir.dt.float32r
AF = mybir.ActivationFunctionType
OP = mybir.AluOpType

# packed input column maps
W32_COLS = 384 + 384 + 3 + 3 + 32 + NCH   # wq | wk | bq | bk | id4 | t2p
WR_COLS = 4 * NCH                         # v4n: per chunk [vr0 vr1 ve0 ve1]


def _s4(xT):
    """(32, N) -> (128, N//4): 128-col chunk c -> partitions 32*(c%4),
    cols 128*(c//4)."""
    d, N = xT.shape
    nch = N // 128
    out = np.zeros((128, N // 4), dtype=xT.dtype)
    for c in range(nch):
        out[32 * (c % 4):32 * (c % 4) + 32,
            128 * (c // 4):128 * (c // 4) + 128] = xT[:, 128 * c:128 * c + 128]
    return out


def _band_meta(t1_all, t2_all):
    w_raw = np.full(NT, T, dtype=np.int64)
    for t1 in t1_all:
        for t2 in t2_all:
            r_min = np.searchsorted(t2, t1[::128], side="right")
            w_raw = np.minimum(w_raw, (r_min // 128) * 128)
    e = np.zeros(NT, dtype=np.int64)
    for t1 in t1_all:
        for t2 in t2_all:
            r_max = np.searchsorted(t2, t1[127::128], side="right")
            e = np.maximum(e, (r_max + 127) // 128)
    wc = w_raw // 128
    e = np.maximum(e, wc + 1)
    e = np.minimum(np.maximum.accumulate(e), NCH)
    wc = np.minimum(wc, e - 1)
    assert np.all(np.diff(wc) >= 0) and np.all(np.diff(e) >= 0)
    for t1 in t1_all:
        for t2 in t2_all:
            r_min = np.searchsorted(t2, t1[::128], side="right")
            r_max = np.searchsorted(t2, t1[127::128], side="right")
            assert np.all(wc * 128 <= r_min) and np.all(r_max <= e * 128)
    return [int(x) for x in wc], [int(x) for x in e]


def _pieces_meta(wc, e, t1_all, t2_all):
    pieces = []
    for c in range(NCH):
        tiles = [I for I in range(NT) if wc[I] <= c < e[I]]
        if not tiles:
            continue
        lo, ihi = tiles[0], tiles[-1] + 1
        qlo = 128 * lo
        qmin = TQ
        for t1 in t1_all:
            for t2 in t2_all:
                qmin = min(qmin, int(np.searchsorted(t1, t2[128 * c])))
        qlo = max(qlo, (qmin // 64) * 64)
        qlo = min(qlo, 128 * ihi - 64)
        pieces.append((c, qlo, lo, ihi))
    return pieces


def _build(wc, e, pieces):
    nc = Bacc("TRN2")

    xk = nc.dram_tensor("xk", [128, T // 4], F32, kind="ExternalInput")
    xq = nc.dram_tensor("xq", [128, TQ // 4], F32, kind="ExternalInput")
    wp32 = nc.dram_tensor("wp32", [128, W32_COLS], F32, kind="ExternalInput")
    wpr = nc.dram_tensor("wpr", [128, WR_COLS], F32R, kind="ExternalInput")
    t1 = nc.dram_tensor("t1", [1, TQ], F32, kind="ExternalInput")
    out = nc.dram_tensor("out", [2, TQ], F32, kind="ExternalOutput")
    hdbg = nc.dram_tensor("hdbg", [32, 2 * NT], F32, kind="ExternalOutput")

    maxw = max(wc)

    gparts = {g: [] for g in range(NG)}
    for idx, (c, qlo, lo, ihi) in enumerate(pieces):
        for g in range((qlo // 512), (ihi * 128 - 1) // 512 + 1):
            a = max(qlo, 512 * g)
            b = min(128 * ihi, 512 * g + 512)
            gparts[g].append((idx, a, b))

    with TileContext(nc) as tc:
        with tc.tile_pool(name="cst", bufs=1) as cst, \
             tc.tile_pool(name="big", bufs=1) as big:

            wp32_s = cst.tile([128, W32_COLS], F32)
            wpr_s = cst.tile([128, WR_COLS], F32R)
            wq_s = wp32_s[:, 0:384]
            wk_s = wp32_s[:, 384:768]
            bq_s = wp32_s[:, 768:771]
            bk_s = wp32_s[:, 771:774]
            id4_s = wp32_s[:, 774:806]
            t2p_s = wp32_s[:, 806:806 + NCH]
            v4n_s = wpr_s[:, 0:4 * NCH]

            t1b_s = big.tile([128, TQ], F32, tag="t1b")
            xk_s = big.tile([128, T // 4], F32, tag="xk")
            xq_s = big.tile([128, TQ // 4], F32, tag="xq")
            kt_s = big.tile([128, T // 4], F32, tag="kt")
            qts32 = big.tile([128, TQ // 4], F32, tag="qts32")
            qrep32 = big.tile([128, TQ], F32, tag="qrep32")
            knr = big.tile([128, 32 * NCH], F32R, tag="knr")
            kne = big.tile([128, 32 * NCH], F32R, tag="kne")
            hsball = cst.tile([32, 2 * NT], F32)
            hsbrep = cst.tile([128, 2 * NT], F32)
            zsb = cst.tile([2, TQ], F32)

            nc.sync.dma_start(wp32_s[:], wp32[:])
            nc.sync.dma_start(xq_s[:], xq[:])
            nc.sync.dma_start(wpr_s[:], wpr[:])
            nc.sync.dma_start(xk_s[:], xk[:])
            nc.gpsimd.dma_start(
                t1b_s[:], t1[0:1, :].partition_broadcast(128))

            # ---------------- MLPs (block-diag 128-contract) -------------
            with tc.tile_pool(name="mlp", bufs=3, space="PSUM") as mlp, \
                 tc.tile_pool(name="hbuf", bufs=2) as hbuf:

                def run_mlp(x_s, w_s, b_s, ngrp, dst, dt):
                    h_prev = x_s
                    for l in range(3):
                        h_next = dst if l == 2 else hbuf.tile(
                            [128, ngrp * 512], dt, tag=f"h{ngrp}",
                            name=f"h{ngrp}_{l}")
                        for G in range(ngrp):
                            pt = mlp.tile([128, 512], F32, tag="mlp")
                            nc.tensor.matmul(
                                pt[:], w_s[:, 128 * l:128 * l + 128],
                                h_prev[:, 512 * G:512 * G + 512],
                                start=True, stop=True)
                            o = h_next[:, 512 * G:512 * G + 512]
                            if G % 2 == 0:
                                if l < 2:
                                    nc.scalar.activation(
                                        o, pt[:], AF.Relu, bias=b_s[:, l:l + 1])
                                else:
                                    nc.scalar.activation(
                                        o, pt[:], AF.Identity,
                                        bias=b_s[:, l:l + 1])
                            else:
                                if l < 2:
                                    nc.vector.tensor_scalar(
                                        o, pt[:], b_s[:, l:l + 1], 0.0,
                                        OP.add, OP.max)
                                else:
                                    nc.vector.tensor_scalar(
                                        o, pt[:], b_s[:, l:l + 1], None,
                                        OP.add)
                        h_prev = h_next

                run_mlp(xq_s, wq_s, bq_s, 2, qts32, F32)
                run_mlp(xk_s, wk_s, bk_s, 4, kt_s, F32)

            # replicated flat Q^T (fp32), issued off-ACT
            for b in range(4):
                for k in range(4):
                    nc.gpsimd.dma_start(
                        qrep32[32 * b:32 * b + 32, :].rearrange(
                            "d (t c) -> d t c", c=128)[:, k::4, :],
                        qts32[32 * k:32 * k + 32, :].rearrange(
                            "d (t c) -> d t c", c=128))

            # knat transposes (fp32) + exact split into knr/kne (fp32r)
            with tc.tile_pool(name="knT", bufs=1, space="PSUM") as knT:
                knrv = knr[:, :].rearrange("p (c d) -> p c d", d=32)
                knev = kne[:, :].rearrange("p (c d) -> p c d", d=32)
                for half in range(2):
                    pts = [knT.tile([128, 256], F32, tag=f"knT{q}",
                                    name=f"knT{q}") for q in range(4)]
                    for i in range(8):
                        for q in range(4):
                            col = 8 * half + i
                            nc.tensor.matmul(
                                pts[q][:, 32 * i:32 * i + 32],
                                kt_s[32 * q:32 * q + 32,
                                     128 * col:128 * col + 128],
                                id4_s[32 * q:32 * q + 32, :],
                                start=True, stop=True,
                                tile_position=(32 * q, 0))
                    for q in range(4):
                        pv = pts[q][:].rearrange("p (c d) -> p c d", d=32)
                        orr = knrv[:, q + 4 * 8 * half::4, :][:, 0:8, :]
                        oe = knev[:, q + 4 * 8 * half::4, :][:, 0:8, :]
                        nc.scalar.activation(orr, pv, AF.Copy)
                        nc.vector.scalar_tensor_tensor(
                            oe, orr.bitcast(F32), -1.0, pv,
                            OP.mult, OP.add)

            # prefix chain (fp32r) + per-tile H snapshots (fp32) -> hsball
            with tc.tile_pool(name="hps", bufs=2, space="PSUM") as hps:
                prev = 0
                for I in range(NT):
                    w = wc[I]
                    dst = hsball[:, 2 * I:2 * I + 2]
                    if w > prev:
                        dps = hps.tile([32, 2], F32, tag="dh")
                        for c in range(prev, w):
                            for li, (kk, v0) in enumerate(
                                    ((knr, 0), (knr, 2), (kne, 0))):
                                nc.tensor.matmul(
                                    dps[:], kk[:, 32 * c:32 * c + 32],
                                    v4n_s[:, 4 * c + v0:4 * c + v0 + 2],
                                    start=(c == prev and li == 0),
                                    stop=(c == w - 1 and li == 2))
                        if I == 0:
                            nc.vector.tensor_copy(dst, dps[:])
                        else:
                            nc.vector.tensor_tensor(
                                dst, hsball[:, 2 * I - 2:2 * I],
                                dps[:], OP.add)
                        prev = w
                    elif I == 0:
                        nc.vector.memset(dst, 0)
                    else:
                        nc.vector.tensor_copy(
                            dst, hsball[:, 2 * I - 2:2 * I])

            # replicate H table (fp32) to all 4 partition blocks
            for q in range(4):
                nc.gpsimd.dma_start(hsbrep[32 * q:32 * q + 32, :], hsball[:])

            # ---------------- band ----------------
            with tc.tile_pool(name="stp", bufs=3, space="PSUM") as stp, \
                 tc.tile_pool(name="zob", bufs=1, space="PSUM") as zob, \
                 tc.tile_pool(name="zop", bufs=1, space="PSUM") as zop, \
                 tc.tile_pool(name="smp", bufs=10) as smp, \
             tc.tile_pool(name="zps", bufs=2) as zpsp:

                # rotated fp32 prefix-zo, 4 groups x 4 tiles per batch
                zoP = {}

                def emit_prefix_batch(gs):
                    for g in gs:
                        zoP[g] = zop.tile([2, 512], F32, tag=f"zoP{g % 4}",
                                          name=f"zoP{g % 4}")
                    for t in range(4):
                        for g in gs:
                            I = 4 * g + t
                            qq = g % 4
                            nc.tensor.matmul(
                                zoP[g][:, 128 * t:128 * t + 128],
                                hsbrep[32 * qq:32 * qq + 32, 2 * I:2 * I + 2],
                                qrep32[32 * qq:32 * qq + 32,
                                       128 * I:128 * I + 128],
                                start=(t == 0), stop=(t == 3),
                                tile_position=(32 * qq, 0),
                                skip_group_check=True)

                emit_prefix_batch(range(0, 4))
                made = {}
                for g in range(NG):
                    if g == 4:
                        emit_prefix_batch(range(4, 8))
                    zoB = zob.tile([4, 512], F32, tag="zoB")
                    for (idx, a, b) in gparts[g]:
                        if idx in made:
                            continue
                        c, qlo, lo, ihi = pieces[idx]
                        wd = 128 * ihi - qlo
                        q = c % 4
                        stb = stp.tile([128, 512], F32, tag="st")
                        nc.tensor.matmul(
                            stb[:, 0:wd],
                            kt_s[32 * q:32 * q + 32,
                                 128 * (c // 4):128 * (c // 4) + 128],
                            qrep32[32 * q:32 * q + 32, qlo:128 * ihi],
                            start=True, stop=True,
                            tile_position=(32 * q, 0))
                        smt = smp.tile([128, 512], F32R, tag="smt")
                        nc.vector.scalar_tensor_tensor(
                            smt[:, 0:wd], t1b_s[:, qlo:128 * ihi],
                            t2p_s[:, c:c + 1], stb[:, 0:wd],
                            OP.is_ge, OP.mult)
                        made[idx] = smt
                    nparts = len(gparts[g])
                    assert nparts > 0
                    for i, (idx, a, b) in enumerate(gparts[g]):
                        c, qlo, lo, ihi = pieces[idx]
                        smt = made[idx]
                        nc.tensor.matmul(
                            zoB[:, a - 512 * g:b - 512 * g],
                            v4n_s[:, 4 * c:4 * c + 4],
                            smt[:, a - qlo:b - qlo],
                            start=(i == 0), stop=(i == nparts - 1),
                            skip_group_check=True)
                    zb4 = zpsp.tile([4, 512], F32, tag="zb4")
                    nc.scalar.activation(zb4[:], zoB[:], AF.Copy)
                    zbs = zpsp.tile([2, 512], F32, tag="zbs")
                    nc.sync.dma_start(zbs[:], zb4[2:4, :])
                    ztmp = zpsp.tile([2, 512], F32, tag="ztmp")
                    nc.vector.tensor_tensor(
                        ztmp[:], zb4[0:2, :], zbs[:], OP.add)
                    nc.vector.tensor_tensor(
                        zsb[:, 512 * g:512 * g + 512], ztmp[:], zoP[g][:],
                        OP.add)

            nc.sync.dma_start(out[:], zsb[:])
            nc.sync.dma_start(hdbg[:], hsball[:])
    nc.finalize()
    return nc


_CACHE = {}
LAST_RESULTS = None


def kernel(m1, m2, m3, m4, Wq, bq, Wk, bk):
    mods = [np.asarray(m)[0, 0].astype(np.float32) for m in (m1, m2, m3, m4)]
    Wq, bq, Wk, bk = (np.asarray(a, dtype=np.float32) for a in (Wq, bq, Wk, bk))
    t2s = [m[:, -1].copy() for m in mods]
    t1g = mods[0][:, -1].copy()

    def qsel(h):
        idx = np.arange(TQ)
        gt = 2 * (idx // 128) + h
        return gt * 128 + (idx % 128)

    sels = [qsel(0), qsel(1)]
    t1_locals = [t1g[s] for s in sels]
    wc, e = _band_meta(t1_locals, t2s)
    pieces = _pieces_meta(wc, e, t1_locals, t2s)

    key = (tuple(wc), tuple(e), tuple(p[1] for p in pieces))
    if key not in _CACHE:
        _CACHE[key] = _build(wc, e, pieces)
    nc = _CACHE[key]

    def chop11(x):
        # zero low 12 mantissa bits: exactly representable in fp32r (11b)
        xm = np.ascontiguousarray(x, np.float32).view(np.uint32)
        return (xm & np.uint32(0xFFFFF000)).view(np.float32).copy()

    def blockdiag(W):
        out = np.zeros((128, 384), dtype=np.float32)
        for l in range(3):
            for k in range(4):
                out[32 * k:32 * k + 32,
                    128 * l + 32 * k:128 * l + 32 * k + 32] = W[l]
        return out

    bq_in = np.tile(bq.T, (4, 1)).astype(np.float32)
    bk_in = np.tile(bk.T, (4, 1)).astype(np.float32)
    id4_in = np.tile(np.eye(32, dtype=np.float32), (4, 1))

    in_maps = []
    for core in range(8):
        mod, h = core // 2, core % 2
        x = mods[mod]
        t2 = t2s[mod]
        xk_in = _s4(np.ascontiguousarray(x.T))
        xq_l = mods[0][sels[h]]
        xq_in = _s4(np.ascontiguousarray(xq_l.T))
        t1_in = t1_locals[h].reshape(1, TQ).astype(np.float32)
        t2p_in = np.ascontiguousarray(t2.reshape(NCH, 128).T)
        v2n_in = np.ascontiguousarray(
            x[:, :2].reshape(NCH, 128, 2).transpose(1, 0, 2)
            .reshape(128, 2 * NCH))
        v2nr_in = chop11(v2n_in)
        v2ne_in = v2n_in - v2nr_in
        v4n_in = np.zeros((128, 4 * NCH), dtype=np.float32)
        v4n_in[:, 0::4] = v2nr_in[:, 0::2]
        v4n_in[:, 1::4] = v2nr_in[:, 1::2]
        v4n_in[:, 2::4] = v2ne_in[:, 0::2]
        v4n_in[:, 3::4] = v2ne_in[:, 1::2]
        wp32_in = np.concatenate(
            [blockdiag(Wq), blockdiag(Wk), bq_in, bk_in, id4_in, t2p_in],
            axis=1)
        wpr_in = v4n_in
        in_maps.append({
            "xk": xk_in, "xq": xq_in, "wp32": wp32_in, "wpr": wpr_in,
            "t1": t1_in,
        })

    import os as _os
    trace = bool(_os.environ.get("KERNEL_TRACE"))
    res = run_bass_kernel_spmd(nc, in_maps, core_ids=list(range(8)),
                               trace=trace)
    global LAST_RESULTS
    LAST_RESULTS = res

    y = np.zeros((T, 2), dtype=np.float32)
    for core in range(8):
        mod, h = core // 2, core % 2
        zt = res.results[core]["out"]
        y[sels[h]] += zt.T
    return y[None, :, :]
